# revision 28
# baseline (speedup 1.0000x reference)
"""Trainium2 Bass kernel for nn_AttentionConv (dense_transformer).

Sharding: data-parallel over batch — 8 NeuronCores, one batch image each.

Per-core dataflow (T=3136 tokens = 56x56, C=384, 6 heads x 64):
  - x shipped pre-transposed from host as xT [C, T] bf16.
  - Q path: depthwise 3x3 conv + BN on DVE+GPSIMD via scalar_tensor_tensor
    tap accumulation in [c, h, w] layout (BN affine folded into tap
    weights/bias on host), then Q projection on PE (softmax scale folded
    into wq on host) -> qh^T [o, T].
  - K/V path: stride-2 depthwise conv + BN + projection FUSED into 9
    shifted PE matmuls per output tile: kh^T = sum_tap Wtap^T @ x^T
    (strided access patterns), accumulated in PSUM. BN bias folded into a
    per-o bias applied at PSUM evacuation.
  - Attention per head: scores^T [t, q] = kh^T.T @ qh^T on PE, exp on ACT
    (no max-subtraction: |scores| << 1 by construction), o^T [65, q] =
    [vh | ones]^T @ e^T accumulated over t tiles (ones column yields the
    softmax denominator as psum row 64). Denominator folded to [W/8, 8]
    via a DRAM bounce, reciprocal on DVE at full lane use, broadcast back
    to 64 partitions via a 0-stride DMA, applied during PSUM evacuation.
  - vh^T produced from vh [o, t] via PE transpose (identity shipped).
  - Output projection in [l, o] orientation (o^T slices stationary),
    result DMA'd straight to DRAM rows. b_last added on host.
"""
import sys

sys.path.insert(0, '/opt/trn_rl_repo')

import numpy as np

DIM = 384
HEADS = 6
D = 64
S = 56           # stride-1 spatial side
S2 = 28          # stride-2 spatial side
T = S * S        # 3136
T2 = S2 * S2     # 784
EPS = 1e-5
SCALE = DIM ** -0.5
NCORES = 8
CT = DIM // 128          # 3 channel tiles
NTT = (T2 + 127) // 128  # 7 kv t-tiles (last = 16 rows)
QB = 1024                # attention q band width
BANDS = [(0, 1024), (1024, 1024), (2048, 1024), (3072, 64)]

TAPS = [(dy, dx) for dy in (-1, 0, 1) for dx in (-1, 0, 1)]  # k=(dy+1)*3+(dx+1)


def build_program():
    import concourse.mybir as mybir
    from concourse import bacc
    from concourse.tile import TileContext

    dt = mybir.dt
    AF = mybir.ActivationFunctionType
    ALU = mybir.AluOpType

    nc = bacc.Bacc()

    SP = S + 2
    xT = nc.dram_tensor("xT", [DIM, SP * SP], dt.bfloat16,
                        kind="ExternalInput")
    qcp = nc.dram_tensor("qcp", [DIM, 10], dt.float32, kind="ExternalInput")
    wqt = nc.dram_tensor("wqt", [DIM, DIM], dt.bfloat16, kind="ExternalInput")
    wkt = nc.dram_tensor("wkt", [9, DIM, DIM], dt.bfloat16, kind="ExternalInput")
    wvt = nc.dram_tensor("wvt", [9, DIM, DIM], dt.bfloat16, kind="ExternalInput")
    kvb = nc.dram_tensor("kvb", [DIM, 2], dt.float32, kind="ExternalInput")
    wlt = nc.dram_tensor("wlt", [DIM, DIM], dt.bfloat16, kind="ExternalInput")
    idin = nc.dram_tensor("idin", [128, 128], dt.bfloat16, kind="ExternalInput")
    out = nc.dram_tensor("out", [T, DIM], dt.float32, kind="ExternalOutput")

    with TileContext(nc) as tc:
        with (
            tc.tile_pool(name="const", bufs=1) as cpool,
            tc.tile_pool(name="work", bufs=1) as wpool,
            tc.tile_pool(name="ework", bufs=3) as epool,
            tc.tile_pool(name="psA", bufs=2, space="PSUM") as psA,
            tc.tile_pool(name="psB", bufs=2, space="PSUM") as psB,
            tc.tile_pool(name="dram", bufs=2, space="DRAM") as dpool,
        ):
            # ---------------- Phase 0: loads ----------------
            # x arrives zero-padded [58, 58] so every conv tap is full-region
            xT_sb = cpool.tile([128, CT, SP, SP], dt.bfloat16)
            qcp_sb = cpool.tile([128, CT, 10], dt.float32)
            wqt_sb = cpool.tile([128, CT, DIM], dt.bfloat16)
            wkt_sb = cpool.tile([128, 9 * CT, DIM], dt.bfloat16)
            wvt_sb = cpool.tile([128, 9 * CT, DIM], dt.bfloat16)
            kvb_sb = cpool.tile([128, CT, 2], dt.float32)
            wlt_sb = cpool.tile([128, CT, DIM], dt.bfloat16)
            ident = cpool.tile([128, 128], dt.bfloat16)

            # load order matches consumption: x + conv params first, then K
            # weights in (ctile, tap) order, then V, then the later-phase
            # weights.
            def csl(c):
                return slice(c * 128, (c + 1) * 128)

            for c in range(CT):
                nc.sync.dma_start(
                    xT_sb[:, c, :, :],
                    xT[csl(c), :].rearrange("p (h w) -> p h w", w=SP))
                nc.sync.dma_start(qcp_sb[:, c, :], qcp[csl(c), :])
            for c in range(CT):
                for k in range(9):
                    nc.sync.dma_start(wkt_sb[:, k * CT + c, :],
                                      wkt[k, csl(c), :])
            for c in range(CT):
                for k in range(9):
                    nc.sync.dma_start(wvt_sb[:, k * CT + c, :],
                                      wvt[k, csl(c), :])
            nc.sync.dma_start(ident[:], idin[:])
            for c in range(CT):
                nc.sync.dma_start(wqt_sb[:, c, :], wqt[csl(c), :])
                nc.sync.dma_start(kvb_sb[:, c, :], kvb[csl(c), :])
                nc.sync.dma_start(wlt_sb[:, c, :], wlt[csl(c), :])

            # persistent activations
            q_feat = cpool.tile([128, CT, T], dt.bfloat16)
            qh_sb = cpool.tile([128, CT, T], dt.bfloat16)
            kh_sb = cpool.tile([128, CT, T2], dt.bfloat16)
            vh_sb = cpool.tile([128, CT, T2], dt.bfloat16)
            vhT_sb = cpool.tile([128, NTT, HEADS * 65], dt.bfloat16)
            o_sb = cpool.tile([128, CT, T], dt.bfloat16)
            den_scr = cpool.tile([128, QB], dt.float32)
            den_fold = cpool.tile([128, QB // 8], dt.float32)
            r_fold = cpool.tile([128, QB // 8], dt.float32)

            # ---- Phase 1: Q depthwise conv + BN, split across three engines
            # (DVE: 5 STT taps; ACT: 3 scaled-copy taps; GPSIMD: pair adds),
            # emitted first so it overlaps the PE-only K/V fused phase below.
            for c in range(CT):
                x3 = xT_sb[:, c, :, :]  # [128, 58, 58] zero-padded
                acc_d = wpool.tile([128, S, S], dt.float32, tag="acc_d")
                t6 = wpool.tile([128, S, S], dt.bfloat16, tag="t6")
                t7 = wpool.tile([128, S, S], dt.bfloat16, tag="t7")
                t8 = wpool.tile([128, S, S], dt.bfloat16, tag="t8")
                acc_g = wpool.tile([128, S, S], dt.bfloat16, tag="acc_g")

                def tap_in(k):
                    dy, dx = TAPS[k]
                    return x3[:, 1 + dy:1 + dy + S, 1 + dx:1 + dx + S]

                for tk, tmp in ((6, t6), (7, t7), (8, t8)):
                    nc.scalar.activation(
                        tmp[:], tap_in(tk), AF.Copy,
                        scale=qcp_sb[:, c, tk:tk + 1])
                nc.gpsimd.tensor_tensor(out=acc_g[:], in0=t6[:], in1=t7[:],
                                        op=ALU.add)
                nc.gpsimd.tensor_tensor(out=acc_g[:], in0=acc_g[:],
                                        in1=t8[:], op=ALU.add)
                nc.vector.tensor_scalar(
                    out=acc_d[:], in0=tap_in(4),
                    scalar1=qcp_sb[:, c, 4:5], scalar2=qcp_sb[:, c, 9:10],
                    op0=ALU.mult, op1=ALU.add)
                for k in (0, 1, 2, 3, 5):
                    nc.vector.scalar_tensor_tensor(
                        out=acc_d[:], in0=tap_in(k),
                        scalar=qcp_sb[:, c, k:k + 1],
                        in1=acc_d[:],
                        op0=ALU.mult, op1=ALU.add)
                nc.vector.tensor_tensor(
                    out=q_feat[:, c, :].rearrange("p (h w) -> p h w", w=S),
                    in0=acc_d[:], in1=acc_g[:], op=ALU.add)

            # ------------- Phase 3: K/V fused conv+proj ([o, T2]) -----------
            def kv_fused(w_sb, dst_sb, bias_col):
                # All taps are full-region thanks to the zero-padded x.
                # rows 2h'+dy+1 of the padded image: dy=-1 -> (h', 0),
                # dy=0 -> (h', 1), dy=+1 -> (h'+1, 0)   (58 = 29*2)
                pairs = [(c, k) for c in range(CT) for k in range(9)]
                for ot in range(CT):
                    osl = slice(ot * 128, (ot + 1) * 128)
                    for ha, hb in ((0, 14), (14, 28)):
                        ps = psA.tile([128, QB], dt.float32, tag="psA")
                        for i, (c, k) in enumerate(pairs):
                            x5 = xT_sb[:, c, :, :].rearrange(
                                "p (h sy) (w sx) -> p h sy w sx", sy=2, sx=2)
                            dy, dx = TAPS[k]
                            hoff, sy = ((0, 0) if dy == -1 else
                                        (0, 1) if dy == 0 else (1, 0))
                            woff, sx = ((0, 0) if dx == -1 else
                                        (0, 1) if dx == 0 else (1, 0))
                            nc.tensor.matmul(
                                ps[:, 0:(hb - ha) * S2],
                                w_sb[:, k * CT + c, osl],
                                x5[:, ha + hoff:hb + hoff, sy,
                                   woff:woff + S2, sx],
                                start=(i == 0), stop=(i == len(pairs) - 1))
                        nc.vector.tensor_scalar_add(
                            dst_sb[:, ot, ha * S2:hb * S2],
                            ps[:, 0:14 * S2],
                            kvb_sb[:, ot, bias_col:bias_col + 1])

            kv_fused(wkt_sb, kh_sb, 0)
            kv_fused(wvt_sb, vh_sb, 1)

            # -------- Phase 4: vh^T [t, (head, 65)] with ones column --------
            v4 = vhT_sb[:].rearrange("p n (h c) -> p n h c", c=65)
            nc.vector.memset(v4[:, :, :, 64:65], 1.0)
            for tt in range(NTT):
                tsz = min(128, T2 - tt * 128)
                for ot in range(CT):
                    pst = psB.tile([128, QB], dt.bfloat16, tag="psB")
                    nc.tensor.transpose(
                        pst[0:tsz, 0:128],
                        vh_sb[:, ot, tt * 128:tt * 128 + tsz],
                        ident[:])
                    nc.vector.tensor_copy(
                        v4[0:tsz, tt, 2 * ot:2 * ot + 2, 0:64],
                        pst[0:tsz, 0:128].rearrange("p (h c) -> p h c", c=64))

            # ---------------- Phase 2: Q projection (qh^T [o, T]) -----------
            LCH = 448  # 7 chunks exactly
            for lc in (0, 1, 2, 3, 4, 5, 6):
                lsl = slice(lc * LCH, (lc + 1) * LCH)
                for ot in range(CT):
                    osl = slice(ot * 128, (ot + 1) * 128)
                    ps = psA.tile([128, QB], dt.float32, tag="psA")
                    for c in range(CT):
                        nc.tensor.matmul(
                            ps[:, 0:LCH], wqt_sb[:, c, osl], q_feat[:, c, lsl],
                            start=(c == 0), stop=(c == CT - 1))
                    nc.vector.tensor_copy(qh_sb[:, ot, lsl], ps[:, 0:LCH])

            # ---------------- Phase 5: attention ----------------
            # band-outer / head-inner; the PREVIOUS band's output-projection
            # tiles are spread between this band's heads so PE fills the
            # ACT-bound stretches without starving the scores PSUM slots.
            def oproj_tile(lpos, lsz):
                ps = psA.tile([128, QB], dt.float32, tag="psA")
                for c in range(CT):
                    nc.tensor.matmul(
                        ps[0:lsz, 0:DIM], o_sb[:, c, lpos:lpos + lsz],
                        wlt_sb[:, c, :],
                        start=(c == 0), stop=(c == CT - 1))
                ostage = epool.tile([128, DIM], dt.float32, tag="ostage")
                nc.vector.tensor_copy(ostage[0:lsz, :], ps[0:lsz, 0:DIM])
                nc.sync.dma_start(out[lpos:lpos + lsz, :], ostage[0:lsz, :])

            def band_ltiles(qs, W):
                return [(qs + i, min(128, qs + W - (qs + i)))
                        for i in range(0, W, 128)]

            prev_band = None
            for qs, W in BANDS:
                for h in range(HEADS):
                    ot = h // 2
                    rb = 64 * (h % 2)
                    hsl = slice(rb, rb + 64)
                    ps_o = psB.tile([128, QB], dt.float32, tag="psB")

                    def scores(tt):
                        tsz = min(128, T2 - tt * 128)
                        ps_s = psA.tile([128, QB], dt.float32, tag="psA")
                        for sub in range(0, W, 512):
                            sw = min(512, W - sub)
                            nc.tensor.matmul(
                                ps_s[0:tsz, sub:sub + sw],
                                kh_sb[hsl, ot, tt * 128:tt * 128 + tsz],
                                qh_sb[hsl, ot, qs + sub:qs + sub + sw],
                                start=True, stop=True)
                        return ps_s

                    # software pipeline: scores(tt+1) issues before o_mm(tt)
                    # so PE never stalls on the ACT exp.
                    ps_s = scores(0)
                    for tt in range(NTT):
                        tsz = min(128, T2 - tt * 128)
                        e = epool.tile([128, QB], dt.bfloat16, tag="e")
                        nc.scalar.activation(e[0:tsz, 0:W], ps_s[0:tsz, 0:W],
                                             AF.Exp)
                        if tt + 1 < NTT:
                            ps_s = scores(tt + 1)
                        for sub in range(0, W, 512):
                            sw = min(512, W - sub)
                            nc.tensor.matmul(
                                ps_o[0:65, sub:sub + sw],
                                vhT_sb[0:tsz, tt, h * 65:h * 65 + 65],
                                e[0:tsz, sub:sub + sw],
                                start=(tt == 0), stop=(tt == NTT - 1))
                    # denominator -> fold -> recip -> broadcast -> evac
                    fw = 8
                    fp = W // fw
                    den_dr = dpool.tile([QB], dt.float32, tag="dd")
                    r_dr = dpool.tile([QB], dt.float32, tag="rd")
                    nc.vector.tensor_copy(den_scr[64:65, 0:W],
                                          ps_o[64:65, 0:W])
                    nc.sync.dma_start(den_dr[None, 0:W],
                                      den_scr[64:65, 0:W])
                    nc.sync.dma_start(
                        den_fold[0:fp, 0:fw],
                        den_dr[0:W].rearrange("(p f) -> p f", f=fw))
                    nc.vector.reciprocal(r_fold[0:fp, 0:fw],
                                         den_fold[0:fp, 0:fw])
                    nc.sync.dma_start(
                        r_dr[0:W].rearrange("(p f) -> p f", f=fw),
                        r_fold[0:fp, 0:fw])
                    r_rep = epool.tile([64, QB], dt.float32, tag="r_rep")
                    nc.sync.dma_start(
                        r_rep[0:64, 0:W],
                        r_dr[None, 0:W].to_broadcast([64, W]))
                    nc.vector.tensor_tensor(
                        out=o_sb[hsl, ot, qs:qs + W],
                        in0=ps_o[0:64, 0:W], in1=r_rep[0:64, 0:W],
                        op=ALU.mult)

                    if prev_band is not None:
                        tiles = band_ltiles(*prev_band)
                        if h < len(tiles):
                            oproj_tile(*tiles[h])

                if prev_band is not None:
                    for lt in band_ltiles(*prev_band)[HEADS:]:
                        oproj_tile(*lt)
                prev_band = (qs, W)

            for lt in band_ltiles(*prev_band):
                oproj_tile(*lt)

    nc.compile()
    return nc


_CACHE = {}


def _prep_weights(inputs):
    import ml_dtypes
    bf16 = ml_dtypes.bfloat16
    f32 = np.float32

    def bn_fold(prefix):
        a = (np.asarray(inputs[f'bn{prefix}_s'], f32)
             / np.sqrt(np.asarray(inputs[f'bn{prefix}_v'], f32) + EPS))
        b = (np.asarray(inputs[f'bn{prefix}_b'], f32)
             - np.asarray(inputs[f'bn{prefix}_m'], f32) * a)
        return a.astype(f32), b.astype(f32)

    aq, bq = bn_fold('q')
    ak, bk = bn_fold('k')
    av, bv = bn_fold('v')

    conv_q = np.asarray(inputs['conv_q'], f32)[:, 0].reshape(DIM, 9)
    conv_k = np.asarray(inputs['conv_k'], f32)[:, 0].reshape(DIM, 9)
    conv_v = np.asarray(inputs['conv_v'], f32)[:, 0].reshape(DIM, 9)
    wq = np.asarray(inputs['wq'], f32)
    wk = np.asarray(inputs['wk'], f32)
    wv = np.asarray(inputs['wv'], f32)
    wl = np.asarray(inputs['w_last'], f32)

    qcp = np.zeros((DIM, 10), f32)
    qcp[:, :9] = conv_q * aq[:, None]
    qcp[:, 9] = bq

    wqt = np.ascontiguousarray((wq * SCALE).T).astype(bf16)  # [c, o]

    wkt = np.empty((9, DIM, DIM), f32)
    wvt = np.empty((9, DIM, DIM), f32)
    for k in range(9):
        wkt[k] = wk.T * (ak * conv_k[:, k])[:, None]
        wvt[k] = wv.T * (av * conv_v[:, k])[:, None]
    wkt = wkt.astype(bf16)
    wvt = wvt.astype(bf16)

    kvb = np.stack([wk @ bk, wv @ bv], axis=1).astype(f32)  # [o, 2]
    wlt = np.ascontiguousarray(wl.T).astype(bf16)
    idin = np.eye(128, dtype=bf16)
    return qcp, wqt, wkt, wvt, kvb, wlt, idin


def _prep_x(xb):
    """[T, C] f32 -> zero-padded transposed [C, 58*58] bf16."""
    import ml_dtypes
    pad = np.zeros((DIM, S + 2, S + 2), np.float32)
    pad[:, 1:1 + S, 1:1 + S] = xb.T.reshape(DIM, S, S)
    return pad.reshape(DIM, (S + 2) * (S + 2)).astype(ml_dtypes.bfloat16)


def kernel(**inputs):
    from concourse.bass_utils import run_bass_kernel_spmd
    import ml_dtypes
    bf16 = ml_dtypes.bfloat16

    if 'nc' not in _CACHE:
        _CACHE['nc'] = build_program()
    nc = _CACHE['nc']

    qcp, wqt, wkt, wvt, kvb, wlt, idin = _prep_weights(inputs)
    x = np.asarray(inputs['x'], np.float32)  # [8, T, C]
    B = x.shape[0]

    in_maps = []
    for b in range(B):
        in_maps.append({
            'xT': _prep_x(x[b]), 'qcp': qcp, 'wqt': wqt, 'wkt': wkt,
            'wvt': wvt, 'kvb': kvb, 'wlt': wlt, 'idin': idin,
        })

    res = run_bass_kernel_spmd(nc, in_maps, list(range(NCORES)))
    outs = np.stack([np.asarray(res.results[b]['out']) for b in range(B)],
                    axis=0)
    outs = outs + np.asarray(inputs['b_last'], np.float32)[None, None, :]
    return outs.astype(np.float32)
